# revision 18
# baseline (speedup 1.0000x reference)
"""Trainium2 Bass kernel for nn_DialogueGraphModel (relational GAT layer).

Self-contained: takes FULL inputs, shards across 8 NeuronCores, runs a
single SPMD Bass/Tile NEFF (with AllGather collectives), returns the
FULL [50000, 128] float32 output.

Design:
- Nodes split contiguously across 8 cores; edges partitioned by target
  chunk (128 targets) so segment softmax/sum stay core-local.
- Features are mask-premultiplied and cast to bf16 on host; pooling is 16
  identity-lhsT accumulating PE matmuls + a speaker one-hot matmul.
- Weight transforms (G-basis fold of W_r/a_src, a_dst fold of W_q,
  W~out = G^-1 W_out) are computed on host.
- The per-(relation,source) message table is built locally and exchanged
  with TWO AllGathers (relation pairs 0/1 and 2/3) so the per-edge row
  gathers for pair 0 overlap the second collective.
- Per-edge rows are fetched with SWDGE dma_gather over 4 queues; edge
  slots are packed per (group, window, pair) cell with shared boundary
  columns to minimize padding.
- Per-edge target scores (sde) come from host-shipped one-hot A^T (fp8)
  lhsT matmuls against per-chunk sdst, overlapping the collectives.
- Segment softmax/sum are one-hot matmuls (A built by DVE is_equal in
  fp8).  All phase-2 DVE ops are tensor_tensor/tensor_reduce forms so
  DVE never enters 2-port mode (which would stall SWDGE descriptor
  generation); leaky-relu/exp/relu run on the scalar engine.
"""
import ml_dtypes
import numpy as np
import orjson

import concourse.bass as bass
import concourse.bacc as bacc
import concourse.mybir as mybir
import concourse.tile as tile
from concourse.masks import make_identity

F32 = mybir.dt.float32
BF16 = mybir.dt.bfloat16
FP8 = mybir.dt.float8e4
I16 = mybir.dt.int16

C = 8
L = 16
D = 128
H = 8
DH = 16
R = 4
NEG_SLOPE = 0.2
N_FULL = 50000
NLP = 6272            # padded nodes per core (49 * 128); 6250 real
NL = N_FULL // C
NT = NLP // 128
GC = 4                # chunks per gather group
NQ = 4                # SWDGE queues for gathers
WIN2 = NLP * 2        # rows per (window, pair): (node, rel%2)


# --------------------------------------------------------------------------
# BIR post-pass: this container's walrus supports only ONE sync wait per
# instruction; Tile emits more.  Split extra waits onto NoOp carriers.
# --------------------------------------------------------------------------

def _split_waits_in_bir(bir_bytes, max_waits=1):
    d = orjson.loads(bir_bytes)
    for fn in d["functions"]:
        for bb in fn["blocks"]:
            out = []
            for inst in bb["instructions"]:
                si = inst.get("sync_info")
                waits = (si or {}).get("on_wait") or []
                if len(waits) > max_waits:
                    extra, keep = waits[:-max_waits], waits[-max_waits:]
                    for j, w in enumerate(extra):
                        out.append({
                            "debug": inst.get("debug", 0),
                            "engine": inst["engine"],
                            "ins": [],
                            "name": f"{inst['name']}-wsplit{j}",
                            "opcode": "NoOp",
                            "outs": [],
                            "sync_info": {"on_update": [], "on_wait": [w]},
                        })
                    si["on_wait"] = keep
                out.append(inst)
            bb["instructions"] = out
    return orjson.dumps(d)


_patched = False


def _install_patch():
    global _patched
    if _patched:
        return
    import concourse.bass2jax as bass2jax
    import concourse.bass_utils as bass_utils
    orig = bass_utils.compile_bir_kernel

    def patched(bir_json, tmpdir, neff_name="file.neff"):
        if isinstance(bir_json, str):
            bir_json = bir_json.encode()
        return orig(_split_waits_in_bir(bir_json), tmpdir, neff_name)

    bass2jax.compile_bir_kernel = patched
    bass_utils.compile_bir_kernel = patched
    _patched = True


# --------------------------------------------------------------------------
# kernel builder
# --------------------------------------------------------------------------

def _build_kernel(colcnt, MAXC, groups, TOT16):
    GTCP = max(max(g["colsH"]) for g in groups)
    GT16 = max(sum(cl["nslots"] for cl in g["cells"]) // 16 for g in groups)

    nc = bacc.Bacc(num_devices=C, num_swdge_queues=NQ)

    feat = nc.dram_tensor("feat", [NLP, L, D], BF16, kind="ExternalInput")
    spk1hT = nc.dram_tensor("spk1hT", [16, NLP], BF16, kind="ExternalInput")
    spk_emb = nc.dram_tensor("spk_emb", [16, D], BF16, kind="ExternalInput")
    Wt4 = nc.dram_tensor("Wt4", [D, R * D], BF16, kind="ExternalInput")
    Wd = nc.dram_tensor("Wd", [D, H], BF16, kind="ExternalInput")
    Wtout = nc.dram_tensor("Wtout", [D, D], BF16, kind="ExternalInput")
    b_out = nc.dram_tensor("b_out", [D], F32, kind="ExternalInput")
    iota_row = nc.dram_tensor("iota_row", [128], BF16, kind="ExternalInput")
    gidx16 = nc.dram_tensor("gidx16", [128, TOT16], I16, kind="ExternalInput")
    tgtloc = nc.dram_tensor("tgtloc", [128, NT * MAXC], BF16,
                            kind="ExternalInput")
    ATH = nc.dram_tensor("ATH", [NT, 128, MAXC * 128], FP8,
                         kind="ExternalInput")

    cc0 = nc.dram_tensor("cc0", [NLP * 2, D], BF16, kind="Internal")
    cc1 = nc.dram_tensor("cc1", [NLP * 2, D], BF16, kind="Internal")
    table0 = nc.dram_tensor("table0", [WIN2 * C, D], BF16, kind="Internal",
                            addr_space="Shared")
    table1 = nc.dram_tensor("table1", [WIN2 * C, D], BF16, kind="Internal",
                            addr_space="Shared")
    y_out = nc.dram_tensor("y", [NLP, D], F32, kind="ExternalOutput")

    def bcast_ap(dram_handle, n, m):
        a = dram_handle[:]
        return bass.AP(tensor=a.tensor, offset=a.offset, ap=[[0, n], [1, m]])

    with tile.TileContext(nc) as tc:
        with tc.tile_pool(name="persist", bufs=1) as pers:
            idf = pers.tile([128, 128], F32)
            make_identity(nc, idf[:])
            idb = pers.tile([128, 128], BF16)
            nc.vector.tensor_copy(out=idb[:], in_=idf[:])
            iota_sb = pers.tile([128, 128], BF16)
            nc.gpsimd.dma_start(out=iota_sb[:], in_=bcast_ap(iota_row, 128, 128))
            bout_sb = pers.tile([128, 128], F32)
            nc.gpsimd.dma_start(out=bout_sb[:], in_=bcast_ap(b_out, 128, 128))
            zo = pers.tile([128, 2], F32)
            nc.vector.memset(zo[:, 0:1], 0.0)
            nc.vector.memset(zo[:, 1:2], 1.0)

            def col_bcast(ap_col, m):
                return bass.AP(tensor=ap_col.tensor, offset=ap_col.offset,
                               ap=[ap_col.ap[0], [0, m]])

            zero_b = col_bcast(zo[:, 0:1], 128)
            one_b = col_bcast(zo[:, 1:2], 128)
            spke_sb = pers.tile([16, D], BF16)
            nc.sync.dma_start(out=spke_sb[:], in_=spk_emb[:, :])
            wt4_sb = pers.tile([128, R * D], BF16)
            nc.sync.dma_start(out=wt4_sb[:], in_=Wt4[:, :])
            wd_sb = pers.tile([128, H], BF16)
            nc.sync.dma_start(out=wd_sb[:], in_=Wd[:, :])
            wtout_sb = pers.tile([128, D], BF16)
            nc.sync.dma_start(out=wtout_sb[:], in_=Wtout[:, :])

            xloc = pers.tile([128, NT, D], BF16, name="xloc")
            sdst = pers.tile([128, NT, H], BF16, name="sdst")
            sde_sb = pers.tile([128, NT, MAXC, H], BF16, name="sde")
            tl_all = pers.tile([128, NT * MAXC], BF16, name="tl")
            nc.sync.dma_start(out=tl_all[:], in_=tgtloc[:, :])

            # phase 0/1 fused: pooling (identity lhsT, host-premultiplied
            # mask) + speaker embedding + projections per 128-node tile.
            cv0 = cc0[:].rearrange("(n r) e -> n r e", r=2)
            cv1 = cc1[:].rearrange("(n r) e -> n r e", r=2)
            with tc.tile_pool(name="p01", bufs=3) as p01, \
                 tc.tile_pool(name="p01ps", bufs=2, space="PSUM") as ps01:
                for c in range(NT):
                    sl = slice(c * 128, (c + 1) * 128)
                    ftb = p01.tile([128, L, D], BF16, tag="ft")
                    eng = nc.sync if c % 2 == 0 else nc.scalar
                    eng2 = nc.scalar if c % 2 == 0 else nc.sync
                    eng.dma_start(out=ftb[:], in_=feat[sl, :, :])
                    sp = p01.tile([16, 128], BF16, tag="sp")
                    eng2.dma_start(out=sp[:], in_=spk1hT[:, sl])
                    psx = ps01.tile([128, D], F32, tag="psx")
                    for l in range(L):
                        nc.tensor.matmul(psx[:], lhsT=idb[:], rhs=ftb[:, l, :],
                                         start=(l == 0), stop=False)
                    nc.tensor.matmul(psx[:], lhsT=sp[:], rhs=spke_sb[:],
                                     start=False, stop=True)
                    nc.vector.tensor_copy(out=xloc[:, c, :], in_=psx[:])
                    pst = ps01.tile([128, 128], BF16, tag="pst")
                    nc.tensor.transpose(out=pst[:], in_=xloc[:, c, :],
                                        identity=idb[:])
                    xT = p01.tile([128, 128], BF16, tag="xT")
                    nc.vector.tensor_copy(out=xT[:], in_=pst[:])
                    h4 = ps01.tile([128, R * D], F32, tag="h4")
                    nc.tensor.matmul(h4[:], lhsT=xT[:], rhs=wt4_sb[:],
                                     start=True, stop=True)
                    t512 = p01.tile([128, R, D], BF16, tag="t512")
                    nc.vector.tensor_copy(
                        out=t512[:], in_=h4[:].rearrange("p (r d) -> p r d", r=R))
                    eng2.dma_start(out=cv0[sl, :, :], in_=t512[:, 0:2, :])
                    eng.dma_start(out=cv1[sl, :, :], in_=t512[:, 2:4, :])
                    psq = ps01.tile([128, H], F32, tag="psq")
                    nc.tensor.matmul(psq[:], lhsT=xT[:], rhs=wd_sb[:],
                                     start=True, stop=True)
                    nc.vector.tensor_copy(out=sdst[:, c, :], in_=psq[:])

            nc.gpsimd.collective_compute(
                "AllGather", mybir.AluOpType.bypass,
                ins=[cc0[:, :]], outs=[table0[:, :]],
                replica_groups=[list(range(C))],
            )
            nc.gpsimd.collective_compute(
                "AllGather", mybir.AluOpType.bypass,
                ins=[cc1[:, :]], outs=[table1[:, :]],
                replica_groups=[list(range(C))],
            )

            # overlapped with the collectives: per-edge target scores
            # sde[e,h] = sdst[tgt(e),h] via host-shipped one-hot A^T lhsT.
            with tc.tile_pool(name="sdep", bufs=3) as sdep, \
                 tc.tile_pool(name="sdeps", bufs=2, space="PSUM") as sdeps:
                for c in range(NT):
                    ncol = colcnt[c]
                    atc = sdep.tile([128, MAXC * 128], FP8, tag="atc")
                    nc.scalar.dma_start(out=atc[:, :ncol * 128],
                                        in_=ATH[c, :, :ncol * 128])
                    psd = sdeps.tile([128, MAXC, H], F32, tag="psd")
                    for j in range(ncol):
                        nc.tensor.matmul(psd[:, j, :],
                                         lhsT=atc[:, j * 128:(j + 1) * 128],
                                         rhs=sdst[:, c, :],
                                         start=True, stop=True)
                    nc.scalar.activation(
                        out=sde_sb[:, c, :ncol, :], in_=psd[:, :ncol, :],
                        func=mybir.ActivationFunctionType.Copy)

            # phase 2: gather + scores + segment softmax + aggregation.
            # pair-0 gathers run one group ahead so they overlap the second
            # AllGather; pair-1 gathers + compute trail by one step.
            with tc.tile_pool(name="p2g0", bufs=3) as p2g0, \
                 tc.tile_pool(name="p2g1", bufs=2) as p2g1, \
                 tc.tile_pool(name="p2gi", bufs=3) as p2gi, \
                 tc.tile_pool(name="p2", bufs=3) as p2, \
                 tc.tile_pool(name="p2w", bufs=2) as p2w, \
                 tc.tile_pool(name="p2s", bufs=4) as p2s, \
                 tc.tile_pool(name="psAgg", bufs=2, space="PSUM") as psAgg, \
                 tc.tile_pool(name="psO", bufs=2, space="PSUM") as psO:
              gt_of = {}
              gi_of = {}

              def emit_gathers(gx, pair):
                  grp = groups[gx]
                  if pair == 0:
                      git = p2gi.tile([128, GT16], I16, tag="gi16")
                      gi_of[gx] = git
                      off0 = grp["cells"][0]["off16"]
                      n16 = sum(cl["nslots"] for cl in grp["cells"]) // 16
                      nc.sync.dma_start(out=git[:, :n16],
                                        in_=gidx16[:, off0:off0 + n16])
                      gt = p2g0.tile([128, GTCP, D], BF16, tag="gt0")
                      gt_of[(gx, 0)] = gt
                  else:
                      git = gi_of[gx]
                      off0 = grp["cells"][0]["off16"]
                      gt = p2g1.tile([128, GTCP, D], BF16, tag="gt1")
                      gt_of[(gx, 1)] = gt
                  tab = table0 if pair == 0 else table1
                  for cl in grp["cells"]:
                      if cl["pair"] != pair or cl["nslots"] == 0:
                          continue
                      l16 = cl["off16"] - off0
                      c0 = cl["col0h"]
                      w = cl["w"]
                      for p0 in range(0, cl["nslots"], 1024):
                          pn = min(1024, cl["nslots"] - p0)
                          nc.gpsimd.dma_gather(
                              gt[:, c0 + p0 // 128:c0 + (p0 + pn) // 128, :],
                              tab[w * WIN2:(w + 1) * WIN2, :],
                              git[:, l16 + p0 // 16:l16 + (p0 + pn) // 16],
                              pn, pn, D, queue_num=(2 * w + pair) % NQ)

              for gx, grp in enumerate(groups):
                if gx == 0:
                    emit_gathers(0, 0)
                if gx + 1 < len(groups):
                    emit_gathers(gx + 1, 0)
                emit_gathers(gx, 1)
                for c in grp["chunks"]:
                    NCOL = colcnt[c]
                    runs = grp["runs"][c]
                    sl = slice(c * 128, (c + 1) * 128)

                    tla = tl_all[:, c * MAXC:c * MAXC + NCOL]
                    tl_b = bass.AP(tensor=tla.tensor, offset=tla.offset,
                                   ap=[tla.ap[0], [1, NCOL], [0, 128]])
                    ioa = iota_sb[:]
                    io_b = bass.AP(tensor=ioa.tensor, offset=ioa.offset,
                                   ap=[ioa.ap[0], [0, NCOL], ioa.ap[1]])
                    A_all = p2.tile([128, MAXC, 128], FP8, tag="A")
                    nc.vector.tensor_tensor(out=A_all[:, :NCOL, :], in0=tl_b,
                                            in1=io_b,
                                            op=mybir.AluOpType.is_equal)

                    # scores: TT add (DVE, never 2-port) + Prelu/Exp on ACT
                    s_all = p2s.tile([128, MAXC, H], F32, tag="sall")
                    for (l0, k, g0, pr) in runs:
                        gtile = gt_of[(gx, pr)]
                        nc.vector.tensor_add(out=s_all[:, l0:l0 + k, :],
                                             in0=gtile[:, g0:g0 + k, 0:H],
                                             in1=sde_sb[:, c, l0:l0 + k, :])
                    nc.scalar.activation(out=s_all[:, :NCOL, :],
                                         in_=s_all[:, :NCOL, :],
                                         func=mybir.ActivationFunctionType.Prelu,
                                         alpha=NEG_SLOPE)
                    ex = p2s.tile([128, MAXC, H], BF16, tag="ex")
                    nc.scalar.activation(out=ex[:, :NCOL, :],
                                         in_=s_all[:, :NCOL, :],
                                         func=mybir.ActivationFunctionType.Exp)

                    wm = p2w.tile([128, MAXC, 136], BF16, tag="wm")
                    for (l0, k, g0, pr) in runs:
                        gtile = gt_of[(gx, pr)]
                        exr = ex[:, l0:l0 + k, :]
                        nc.vector.tensor_tensor(out=wm[:, l0:l0 + k, 0:H],
                                                in0=gtile[:, g0:g0 + k, 0:H],
                                                in1=exr,
                                                op=mybir.AluOpType.mult)
                        eb = bass.AP(tensor=exr.tensor, offset=exr.offset,
                                     ap=[exr.ap[0], exr.ap[1], exr.ap[2],
                                         [0, 15]])
                        nc.vector.tensor_tensor(
                            out=wm[:, l0:l0 + k, H:D].rearrange(
                                "p j (h k2) -> p j h k2", h=H),
                            in0=gtile[:, g0:g0 + k, H:D].rearrange(
                                "p j (h k2) -> p j h k2", h=H),
                            in1=eb, op=mybir.AluOpType.mult)
                    nc.scalar.activation(out=wm[:, :NCOL, D:136],
                                         in_=s_all[:, :NCOL, :],
                                         func=mybir.ActivationFunctionType.Exp)

                    agg = psAgg.tile([128, 136], F32, tag="agg")
                    for j in range(NCOL):
                        nc.tensor.matmul(agg[:], lhsT=A_all[:, j, :],
                                         rhs=wm[:, j, :],
                                         start=(j == 0), stop=(j == NCOL - 1))

                    rec2 = p2s.tile([128, H], F32, tag="rec2")
                    nc.vector.reciprocal(out=rec2[:], in_=agg[:, D:136])
                    aggN = p2s.tile([128, D], F32, tag="aggN")
                    nc.vector.tensor_tensor(out=aggN[:, 0:H], in0=agg[:, 0:H],
                                            in1=rec2[:],
                                            op=mybir.AluOpType.mult)
                    ra = rec2[:]
                    r_b = bass.AP(tensor=ra.tensor, offset=ra.offset,
                                  ap=[ra.ap[0], ra.ap[1], [0, 15]])
                    nc.vector.tensor_tensor(
                        out=aggN[:, H:D].rearrange("p (h k) -> p h k", h=H),
                        in0=agg[:, H:D].rearrange("p (h k) -> p h k", h=H),
                        in1=r_b, op=mybir.AluOpType.mult)

                    ps_at2 = psO.tile([128, 128], F32, tag="psat2")
                    nc.tensor.transpose(out=ps_at2[:], in_=aggN[:],
                                        identity=idf[:])
                    at2 = p2s.tile([128, 128], BF16, tag="at2")
                    nc.scalar.activation(out=at2[:], in_=ps_at2[:],
                                         func=mybir.ActivationFunctionType.Copy)
                    ps_o = psO.tile([128, 128], F32, tag="pso")
                    nc.tensor.matmul(ps_o[:], lhsT=at2[:], rhs=wtout_sb[:],
                                     start=True, stop=True)
                    # ELU via TT (single-port DVE) + ACT only.
                    u = p2s.tile([128, 128], F32, tag="u")
                    nc.vector.tensor_add(out=u[:], in0=ps_o[:],
                                         in1=xloc[:, c, :])
                    nc.vector.tensor_add(out=u[:], in0=u[:], in1=bout_sb[:])
                    ng = p2s.tile([128, 128], F32, tag="ng")
                    nc.vector.tensor_tensor(out=ng[:], in0=u[:], in1=zero_b,
                                            op=mybir.AluOpType.min)
                    nc.scalar.activation(out=ng[:], in_=ng[:],
                                         func=mybir.ActivationFunctionType.Exp)
                    ps2 = p2s.tile([128, 128], F32, tag="ps2")
                    nc.scalar.activation(out=ps2[:], in_=u[:],
                                         func=mybir.ActivationFunctionType.Relu)
                    nc.vector.tensor_add(out=ps2[:], in0=ps2[:], in1=ng[:])
                    nc.vector.tensor_tensor(out=ps2[:], in0=ps2[:], in1=one_b,
                                            op=mybir.AluOpType.subtract)
                    eng = nc.sync if c % 2 == 0 else nc.scalar
                    eng.dma_start(out=y_out[sl, :], in_=ps2[:])

    nc.finalize()
    return nc


# --------------------------------------------------------------------------
# host-side prep
# --------------------------------------------------------------------------

def _host_weights(W_r, W_q, a_src, a_dst, W_out):
    d_h = np.abs(a_src).argmax(axis=1)
    kept = [(h, dh) for h in range(H) for dh in range(DH) if dh != d_h[h]]
    G = np.zeros((D, D), np.float64)
    for h in range(H):
        G[h * DH:(h + 1) * DH, h] = a_src[h]
    for jp, (h, dh) in enumerate(kept):
        G[DH * h + dh, 8 + jp] = 1.0
    Wt4 = np.concatenate([W_r[r].astype(np.float64) @ G for r in range(R)],
                         axis=1)
    Wtout = np.linalg.inv(G) @ W_out.astype(np.float64)
    Wd = np.zeros((D, H), np.float64)
    for h in range(H):
        Wd[:, h] = W_q[:, h * DH:(h + 1) * DH].astype(np.float64) @ a_dst[h]
    bf = ml_dtypes.bfloat16
    return Wt4.astype(bf), Wtout.astype(bf), Wd.astype(bf)


def _prep_inputs(inputs):
    feats = np.asarray(inputs["features"], np.float32)
    mask = np.asarray(inputs["mask"])
    edge_index = np.asarray(inputs["edge_index"], np.int64)
    edge_type = np.asarray(inputs["edge_type"], np.int64)
    speaker_ids = np.asarray(inputs["speaker_ids"], np.int64)
    spk_emb = np.asarray(inputs["spk_emb"], np.float32)
    W_r = np.asarray(inputs["W_r"], np.float32)
    W_q = np.asarray(inputs["W_q"], np.float32)
    a_src = np.asarray(inputs["a_src"], np.float32)
    a_dst = np.asarray(inputs["a_dst"], np.float32)
    W_out = np.asarray(inputs["W_out"], np.float32)
    b_out = np.asarray(inputs["b_out"], np.float32)
    bf = ml_dtypes.bfloat16

    Wt4, Wtout, Wd = _host_weights(W_r, W_q, a_src, a_dst, W_out)
    iota_row = np.arange(128, dtype=np.float32).astype(bf)
    spk_emb_b = spk_emb.astype(bf)

    cnt = np.clip(mask.astype(np.float32).sum(1), 1.0, None)
    w_tok = mask.astype(np.float32) / cnt[:, None]
    fw = (feats * w_tok[:, :, None]).astype(bf)

    src, tgt = edge_index[0], edge_index[1]
    rel = edge_type
    core_of = tgt // NL
    loc_t = tgt - core_of * NL
    chunk_of = loc_t // 128
    t_in_chunk = loc_t % 128
    src_core = src // NL
    pairq = rel // 2
    rowp = (src % NL) * 2 + (rel % 2)

    key = (((core_of * NT + chunk_of) * C + src_core) * 2 + pairq)
    order = np.argsort(key, kind="stable")
    key_s = key[order]
    row_s = rowp[order]
    tic_s = t_in_chunk[order]

    NBINS = C * NT * C * 2
    counts = np.bincount(key_s, minlength=NBINS).reshape(C, NT, C, 2)
    seg = counts.max(axis=0)                    # [NT, C, 2] static seg sizes
    starts = np.zeros(NBINS + 1, np.int64)
    np.cumsum(np.bincount(key_s, minlength=NBINS), out=starts[1:])

    # group sizes taper at the end so the pipeline drains faster
    sizes = []
    left = NT
    while left > GC + 2:
        sizes.append(GC)
        left -= GC
    while left > 0:
        s = max(1, min(2, left))
        sizes.append(s)
        left -= s
    bounds = np.concatenate([[0], np.cumsum(sizes)]).astype(int)

    groups = []
    colcnt = [0] * NT
    off16_total = 0
    for gx in range(len(sizes)):
        cs = list(range(bounds[gx], bounds[gx + 1]))
        cells = []
        colsH = [0, 0]
        runs = {c: [] for c in cs}
        for pair in (0, 1):
            for w in range(C):
                segs = [int(seg[c, w, pair]) for c in cs]
                tot = sum(segs)
                nslots = ((tot + 127) // 128) * 128
                segoff = {}
                so = 0
                for c, sg_ in zip(cs, segs):
                    if sg_ > 0:
                        a = so // 128
                        b = (so + sg_ + 127) // 128
                        runs[c].append((colcnt[c], b - a,
                                        colsH[pair] + a, pair))
                        segoff[c] = (so, colcnt[c], a)
                        colcnt[c] += b - a
                    so += sg_
                cells.append(dict(pair=pair, w=w, col0h=colsH[pair],
                                  nslots=nslots, off16=off16_total,
                                  segoff=segoff))
                colsH[pair] += nslots // 128
                off16_total += nslots // 16
        groups.append(dict(chunks=cs, cells=cells, colsH=colsH, runs=runs))
    TOT16 = off16_total
    MAXC = max(colcnt)

    in_maps = []
    for ci in range(C):
        tl = np.full((NT, 128, MAXC), -1.0, np.float32)
        AT = np.zeros((NT, 128, MAXC * 128), np.uint8)
        gidx16 = np.zeros((128, TOT16), np.int16)
        for grp in groups:
            for cl in grp["cells"]:
                if cl["nslots"] == 0:
                    continue
                vals = np.zeros(cl["nslots"], np.int64)
                for c, (so, lc0, a) in cl["segoff"].items():
                    b_ = ((ci * NT + c) * C + cl["w"]) * 2 + cl["pair"]
                    lo, hi = starts[b_], starts[b_ + 1]
                    n = hi - lo
                    q = so + np.arange(n)
                    vals[q] = row_s[lo:hi]
                    prt = q % 128
                    lcol = lc0 + (q // 128 - a)
                    tic = tic_s[lo:hi]
                    tl[c, prt, lcol] = tic
                    AT[c, tic, lcol * 128 + prt] = 0x38  # fp8e4m3 1.0
                i = np.arange(cl["nslots"])
                v16 = vals.astype(np.int16)
                o16 = cl["off16"]
                for kq in range(8):
                    gidx16[i % 16 + 16 * kq, o16 + i // 16] = v16

        sl = slice(ci * NL, (ci + 1) * NL)
        featc = np.zeros((NLP, L, D), bf)
        featc[:NL] = fw[sl]
        spk1hT = np.zeros((16, NLP), bf)
        spk1hT[speaker_ids[sl], np.arange(NL)] = 1.0
        tlT = np.ascontiguousarray(
            tl.transpose(1, 0, 2).reshape(128, NT * MAXC)).astype(bf)

        in_maps.append(dict(
            feat=featc, spk1hT=spk1hT, spk_emb=spk_emb_b,
            Wt4=Wt4, Wd=Wd, Wtout=Wtout, b_out=b_out,
            iota_row=iota_row, gidx16=gidx16, tgtloc=tlT,
            ATH=AT.view(ml_dtypes.float8_e4m3fn),
        ))
    return in_maps, colcnt, MAXC, groups, TOT16


# --------------------------------------------------------------------------
# entry point
# --------------------------------------------------------------------------

def kernel(**inputs):
    _install_patch()
    import concourse.bass2jax as bass2jax

    in_maps, colcnt, MAXC, groups, TOT16 = _prep_inputs(inputs)
    nc = _build_kernel(colcnt, MAXC, groups, TOT16)
    results = bass2jax.run_bass_via_pjrt(nc, in_maps, n_cores=C)
    N = np.asarray(inputs["features"]).shape[0]
    NLc = N // C
    return np.concatenate(
        [np.asarray(results[c]["y"])[:NLc] for c in range(C)], axis=0
    ).astype(np.float32)


# revision 19
# speedup vs baseline: 1.2463x; 1.2463x over previous
"""Trainium2 Bass kernel for nn_DialogueGraphModel (relational GAT layer).

Self-contained: takes FULL inputs, shards across 8 NeuronCores, runs a
single SPMD Bass/Tile NEFF (with an AllGather collective), returns the
FULL [50000, 128] float32 output.

Design:
- Nodes split contiguously across 8 cores; edges partitioned by target
  chunk (128 targets) so segment softmax/sum stay core-local.
- Features are mask-premultiplied and cast to bf16 on host; pooling is 16
  identity-lhsT accumulating PE matmuls + a speaker one-hot matmul.
- Weight transforms (G-basis fold of W_r/a_src, a_dst fold of W_q,
  W~out = G^-1 W_out) are computed on host.
- The per-(relation,source) message table is AllGathered (bf16, Shared
  output) and per-edge rows are fetched with SWDGE dma_gather spread over
  4 queues, double-split per window half for finer overlap.
- Per-edge target scores (sde) come from host-shipped one-hot A^T (fp8)
  lhsT matmuls against per-chunk sdst, overlapping the collective.
- Segment softmax/sum are one-hot matmuls (A built by DVE is_equal in
  fp8).  All phase-2 DVE ops are tensor_tensor/tensor_reduce forms so
  DVE never enters 2-port mode (which would stall SWDGE descriptor
  generation); leaky-relu/exp/relu run on the scalar engine.
"""
import ml_dtypes
import numpy as np
import orjson

import concourse.bass as bass
import concourse.bacc as bacc
import concourse.mybir as mybir
import concourse.tile as tile
from concourse.masks import make_identity

F32 = mybir.dt.float32
BF16 = mybir.dt.bfloat16
FP8 = mybir.dt.float8e4
I16 = mybir.dt.int16

C = 8
L = 16
D = 128
H = 8
DH = 16
R = 4
NEG_SLOPE = 0.2
N_FULL = 50000
NLP = 6272            # padded nodes per core (49 * 128); 6250 real
NL = N_FULL // C
NT = NLP // 128
GC = 4                # chunks per gather group
NQ = 4                # SWDGE queues for gathers


# --------------------------------------------------------------------------
# BIR post-pass: this container's walrus supports only ONE sync wait per
# instruction; Tile emits more.  Split extra waits onto NoOp carriers.
# --------------------------------------------------------------------------

def _split_waits_in_bir(bir_bytes, max_waits=1):
    d = orjson.loads(bir_bytes)
    for fn in d["functions"]:
        for bb in fn["blocks"]:
            out = []
            for inst in bb["instructions"]:
                si = inst.get("sync_info")
                waits = (si or {}).get("on_wait") or []
                if len(waits) > max_waits:
                    extra, keep = waits[:-max_waits], waits[-max_waits:]
                    for j, w in enumerate(extra):
                        out.append({
                            "debug": inst.get("debug", 0),
                            "engine": inst["engine"],
                            "ins": [],
                            "name": f"{inst['name']}-wsplit{j}",
                            "opcode": "NoOp",
                            "outs": [],
                            "sync_info": {"on_update": [], "on_wait": [w]},
                        })
                    si["on_wait"] = keep
                out.append(inst)
            bb["instructions"] = out
    return orjson.dumps(d)


_patched = False


def _install_patch():
    global _patched
    if _patched:
        return
    import concourse.bass2jax as bass2jax
    import concourse.bass_utils as bass_utils
    orig = bass_utils.compile_bir_kernel

    def patched(bir_json, tmpdir, neff_name="file.neff"):
        if isinstance(bir_json, str):
            bir_json = bir_json.encode()
        return orig(_split_waits_in_bir(bir_json), tmpdir, neff_name)

    bass2jax.compile_bir_kernel = patched
    bass_utils.compile_bir_kernel = patched
    _patched = True


# --------------------------------------------------------------------------
# kernel builder
# --------------------------------------------------------------------------

def _runs_of(cm):
    runs = []
    i = 0
    while i < len(cm):
        j = i
        while j + 1 < len(cm) and cm[j + 1] == cm[j] + 1:
            j += 1
        runs.append((i, j - i + 1, cm[i]))
        i = j + 1
    return runs


def _build_kernel(colcnt, MAXC, groups, TOT16):
    NPALL = NLP * C
    WIN = NLP * R                     # table rows per source-core window
    GTC = max(max(g["wmeta"][4][0], g["total_cols"] - g["wmeta"][4][0])
              for g in groups)
    GT16 = max(sum(w[1] for w in g["wmeta"]) // 16 for g in groups)

    nc = bacc.Bacc(num_devices=C, num_swdge_queues=NQ)

    feat = nc.dram_tensor("feat", [NLP, L, D], BF16, kind="ExternalInput")
    spk1hT = nc.dram_tensor("spk1hT", [16, NLP], BF16, kind="ExternalInput")
    spk_emb = nc.dram_tensor("spk_emb", [16, D], BF16, kind="ExternalInput")
    Wt4 = nc.dram_tensor("Wt4", [D, R * D], BF16, kind="ExternalInput")
    Wd = nc.dram_tensor("Wd", [D, H], BF16, kind="ExternalInput")
    Wtout = nc.dram_tensor("Wtout", [D, D], BF16, kind="ExternalInput")
    b_out = nc.dram_tensor("b_out", [D], F32, kind="ExternalInput")
    iota_row = nc.dram_tensor("iota_row", [128], BF16, kind="ExternalInput")
    gidx16 = nc.dram_tensor("gidx16", [128, TOT16], I16, kind="ExternalInput")
    tgtloc = nc.dram_tensor("tgtloc", [128, NT * MAXC], BF16,
                            kind="ExternalInput")
    ATH = nc.dram_tensor("ATH", [NT, 128, MAXC * 128], FP8,
                         kind="ExternalInput")

    cc_in = nc.dram_tensor("cc_in", [NLP * R, D], BF16, kind="Internal")
    table = nc.dram_tensor("table", [NPALL * R, D], BF16, kind="Internal",
                           addr_space="Shared")
    y_out = nc.dram_tensor("y", [NLP, D], F32, kind="ExternalOutput")

    def bcast_ap(dram_handle, n, m):
        a = dram_handle[:]
        return bass.AP(tensor=a.tensor, offset=a.offset, ap=[[0, n], [1, m]])

    with tile.TileContext(nc) as tc:
        with tc.tile_pool(name="persist", bufs=1) as pers:
            idf = pers.tile([128, 128], F32)
            make_identity(nc, idf[:])
            idb = pers.tile([128, 128], BF16)
            nc.vector.tensor_copy(out=idb[:], in_=idf[:])
            iota_sb = pers.tile([128, 128], BF16)
            nc.gpsimd.dma_start(out=iota_sb[:], in_=bcast_ap(iota_row, 128, 128))
            bout_sb = pers.tile([128, 128], F32)
            nc.gpsimd.dma_start(out=bout_sb[:], in_=bcast_ap(b_out, 128, 128))
            zo = pers.tile([128, 2], F32)
            nc.vector.memset(zo[:, 0:1], 0.0)
            nc.vector.memset(zo[:, 1:2], 1.0)

            def col_bcast(ap_col, m):
                return bass.AP(tensor=ap_col.tensor, offset=ap_col.offset,
                               ap=[ap_col.ap[0], [0, m]])

            zero_b = col_bcast(zo[:, 0:1], 128)
            one_b = col_bcast(zo[:, 1:2], 128)
            spke_sb = pers.tile([16, D], BF16)
            nc.sync.dma_start(out=spke_sb[:], in_=spk_emb[:, :])
            wt4_sb = pers.tile([128, R * D], BF16)
            nc.sync.dma_start(out=wt4_sb[:], in_=Wt4[:, :])
            wd_sb = pers.tile([128, H], BF16)
            nc.sync.dma_start(out=wd_sb[:], in_=Wd[:, :])
            wtout_sb = pers.tile([128, D], BF16)
            nc.sync.dma_start(out=wtout_sb[:], in_=Wtout[:, :])

            xloc = pers.tile([128, NT, D], F32, name="xloc")
            sdst = pers.tile([128, NT, H], BF16, name="sdst")
            sde_sb = pers.tile([128, NT, MAXC, H], BF16, name="sde")
            tl_all = pers.tile([128, NT * MAXC], BF16, name="tl")
            nc.sync.dma_start(out=tl_all[:], in_=tgtloc[:, :])

            # phase 0/1 fused: pooling (identity lhsT, host-premultiplied
            # mask) + speaker embedding + projections per 128-node tile.
            cv = cc_in[:].rearrange("(n r) e -> n r e", r=R)
            with tc.tile_pool(name="p01", bufs=3) as p01, \
                 tc.tile_pool(name="p01ps", bufs=2, space="PSUM") as ps01:
                for c in range(NT):
                    sl = slice(c * 128, (c + 1) * 128)
                    ftb = p01.tile([128, L, D], BF16, tag="ft")
                    eng = nc.sync if c % 2 == 0 else nc.scalar
                    eng2 = nc.scalar if c % 2 == 0 else nc.sync
                    eng.dma_start(out=ftb[:], in_=feat[sl, :, :])
                    sp = p01.tile([16, 128], BF16, tag="sp")
                    eng2.dma_start(out=sp[:], in_=spk1hT[:, sl])
                    psx = ps01.tile([128, D], F32, tag="psx")
                    for l in range(L):
                        nc.tensor.matmul(psx[:], lhsT=idb[:], rhs=ftb[:, l, :],
                                         start=(l == 0), stop=False)
                    nc.tensor.matmul(psx[:], lhsT=sp[:], rhs=spke_sb[:],
                                     start=False, stop=True)
                    nc.vector.tensor_copy(out=xloc[:, c, :], in_=psx[:])
                    pst = ps01.tile([128, 128], F32, tag="pst")
                    nc.tensor.transpose(out=pst[:], in_=xloc[:, c, :],
                                        identity=idf[:])
                    xT = p01.tile([128, 128], BF16, tag="xT")
                    nc.vector.tensor_copy(out=xT[:], in_=pst[:])
                    h4 = ps01.tile([128, R * D], F32, tag="h4")
                    nc.tensor.matmul(h4[:], lhsT=xT[:], rhs=wt4_sb[:],
                                     start=True, stop=True)
                    t512 = p01.tile([128, R, D], BF16, tag="t512")
                    nc.vector.tensor_copy(
                        out=t512[:], in_=h4[:].rearrange("p (r d) -> p r d", r=R))
                    eng2.dma_start(out=cv[sl, :, :], in_=t512[:])
                    psq = ps01.tile([128, H], F32, tag="psq")
                    nc.tensor.matmul(psq[:], lhsT=xT[:], rhs=wd_sb[:],
                                     start=True, stop=True)
                    nc.vector.tensor_copy(out=sdst[:, c, :], in_=psq[:])

            nc.gpsimd.collective_compute(
                "AllGather", mybir.AluOpType.bypass,
                ins=[cc_in[:, :]], outs=[table[:, :]],
                replica_groups=[list(range(C))],
            )

            # overlapped with the AllGather: per-edge target scores
            # sde[e,h] = sdst[tgt(e),h] via host-shipped one-hot A^T lhsT.
            with tc.tile_pool(name="sdep", bufs=3) as sdep, \
                 tc.tile_pool(name="sdeps", bufs=2, space="PSUM") as sdeps:
                for c in range(NT):
                    ncol = colcnt[c]
                    atc = sdep.tile([128, MAXC * 128], FP8, tag="atc")
                    nc.scalar.dma_start(out=atc[:, :ncol * 128],
                                        in_=ATH[c, :, :ncol * 128])
                    psd = sdeps.tile([128, MAXC, H], F32, tag="psd")
                    for j in range(ncol):
                        nc.tensor.matmul(psd[:, j, :],
                                         lhsT=atc[:, j * 128:(j + 1) * 128],
                                         rhs=sdst[:, c, :],
                                         start=True, stop=True)
                    nc.scalar.activation(
                        out=sde_sb[:, c, :ncol, :], in_=psd[:, :ncol, :],
                        func=mybir.ActivationFunctionType.Copy)

            # phase 2: gather + scores + segment softmax + aggregation
            with tc.tile_pool(name="p2g", bufs=3) as p2g, \
                 tc.tile_pool(name="p2", bufs=2) as p2, \
                 tc.tile_pool(name="p2s", bufs=4) as p2s, \
                 tc.tile_pool(name="psAgg", bufs=2, space="PSUM") as psAgg, \
                 tc.tile_pool(name="psO", bufs=2, space="PSUM") as psO:
              for grp in groups:
                colsA = grp["wmeta"][4][0]
                gtA = p2g.tile([128, GTC, D], BF16, tag="gtA")
                gtB = p2g.tile([128, GTC, D], BF16, tag="gtB")
                gi16 = p2g.tile([128, GT16], I16, tag="gi16")
                g_off16 = grp["wmeta"][0][2]
                g_n16 = sum(w[1] for w in grp["wmeta"]) // 16
                nc.sync.dma_start(out=gi16[:, :g_n16],
                                  in_=gidx16[:, g_off16:g_off16 + g_n16])

                def gcol(g0):
                    return (gtA, g0) if g0 < colsA else (gtB, g0 - colsA)

                for w in range(C):
                    col0, n_idx, off16 = grp["wmeta"][w]
                    l16 = off16 - g_off16
                    gtile, lc0 = gcol(col0)
                    # ucode caps one gather at 1024 descriptors
                    for p0 in range(0, n_idx, 1024):
                        pn = min(1024, n_idx - p0)
                        nc.gpsimd.dma_gather(
                            gtile[:, lc0 + p0 // 128:
                                  lc0 + (p0 + pn) // 128, :],
                            table[w * WIN:(w + 1) * WIN, :],
                            gi16[:, l16 + p0 // 16:l16 + (p0 + pn) // 16],
                            pn, pn, D, queue_num=w % NQ)

                # A one-hots depend only on host data — they overlap with
                # the collective / gathers.
                A_of = {}
                for c in grp["chunks"]:
                    NCOL = colcnt[c]
                    tla = tl_all[:, c * MAXC:c * MAXC + NCOL]
                    tl_b = bass.AP(tensor=tla.tensor, offset=tla.offset,
                                   ap=[tla.ap[0], [1, NCOL], [0, 128]])
                    ioa = iota_sb[:]
                    io_b = bass.AP(tensor=ioa.tensor, offset=ioa.offset,
                                   ap=[ioa.ap[0], [0, NCOL], ioa.ap[1]])
                    A_all = p2.tile([128, MAXC, 128], FP8, tag=f"A{c % GC}")
                    nc.vector.tensor_tensor(out=A_all[:, :NCOL, :], in0=tl_b,
                                            in1=io_b,
                                            op=mybir.AluOpType.is_equal)
                    A_of[c] = A_all

                for c in grp["chunks"]:
                    NCOL = colcnt[c]
                    runs = []
                    for (l0, k, g0) in _runs_of(grp["cmap"][c]):
                        if g0 < colsA < g0 + k:     # run crosses the A/B split
                            ks = colsA - g0
                            runs += [(l0, ks, g0), (l0 + ks, k - ks, colsA)]
                        else:
                            runs.append((l0, k, g0))
                    sl = slice(c * 128, (c + 1) * 128)
                    A_all = A_of[c]

                    # scores: TT add (DVE, never 2-port) + Prelu/Exp on ACT
                    # — keeps DVE out of 2-port mode so SWDGE gather
                    # descriptor generation is never starved.
                    s_all = p2s.tile([128, MAXC, H], F32, tag="sall")
                    for (l0, k, g0) in runs:
                        gtile, lc = gcol(g0)
                        nc.vector.tensor_add(out=s_all[:, l0:l0 + k, :],
                                             in0=gtile[:, lc:lc + k, 0:H],
                                             in1=sde_sb[:, c, l0:l0 + k, :])
                    nc.scalar.activation(out=s_all[:, :NCOL, :],
                                         in_=s_all[:, :NCOL, :],
                                         func=mybir.ActivationFunctionType.Prelu,
                                         alpha=NEG_SLOPE)
                    ex = p2s.tile([128, MAXC, H], BF16, tag="ex")
                    nc.scalar.activation(out=ex[:, :NCOL, :],
                                         in_=s_all[:, :NCOL, :],
                                         func=mybir.ActivationFunctionType.Exp)

                    wm = p2.tile([128, MAXC, 136], BF16, tag="wm")
                    for (l0, k, g0) in runs:
                        gtile, lc = gcol(g0)
                        exr = ex[:, l0:l0 + k, :]
                        nc.vector.tensor_tensor(out=wm[:, l0:l0 + k, 0:H],
                                                in0=gtile[:, lc:lc + k, 0:H],
                                                in1=exr,
                                                op=mybir.AluOpType.mult)
                        eb = bass.AP(tensor=exr.tensor, offset=exr.offset,
                                     ap=[exr.ap[0], exr.ap[1], exr.ap[2],
                                         [0, 15]])
                        nc.vector.tensor_tensor(
                            out=wm[:, l0:l0 + k, H:D].rearrange(
                                "p j (h k2) -> p j h k2", h=H),
                            in0=gtile[:, lc:lc + k, H:D].rearrange(
                                "p j (h k2) -> p j h k2", h=H),
                            in1=eb, op=mybir.AluOpType.mult)
                    nc.scalar.activation(out=wm[:, :NCOL, D:136],
                                         in_=s_all[:, :NCOL, :],
                                         func=mybir.ActivationFunctionType.Exp)

                    agg = psAgg.tile([128, 136], F32, tag="agg")
                    for j in range(NCOL):
                        nc.tensor.matmul(agg[:], lhsT=A_all[:, j, :],
                                         rhs=wm[:, j, :],
                                         start=(j == 0), stop=(j == NCOL - 1))

                    # eps dropped: targets with zero in-degree only occur in
                    # padded rows, which are discarded on host.
                    rec2 = p2s.tile([128, H], F32, tag="rec2")
                    nc.vector.reciprocal(out=rec2[:], in_=agg[:, D:136])
                    aggN = p2s.tile([128, D], F32, tag="aggN")
                    nc.vector.tensor_tensor(out=aggN[:, 0:H], in0=agg[:, 0:H],
                                            in1=rec2[:],
                                            op=mybir.AluOpType.mult)
                    ra = rec2[:]
                    r_b = bass.AP(tensor=ra.tensor, offset=ra.offset,
                                  ap=[ra.ap[0], ra.ap[1], [0, 15]])
                    nc.vector.tensor_tensor(
                        out=aggN[:, H:D].rearrange("p (h k) -> p h k", h=H),
                        in0=agg[:, H:D].rearrange("p (h k) -> p h k", h=H),
                        in1=r_b, op=mybir.AluOpType.mult)

                    ps_at2 = psO.tile([128, 128], F32, tag="psat2")
                    nc.tensor.transpose(out=ps_at2[:], in_=aggN[:],
                                        identity=idf[:])
                    at2 = p2s.tile([128, 128], BF16, tag="at2")
                    nc.scalar.activation(out=at2[:], in_=ps_at2[:],
                                         func=mybir.ActivationFunctionType.Copy)
                    ps_o = psO.tile([128, 128], F32, tag="pso")
                    nc.tensor.matmul(ps_o[:], lhsT=at2[:], rhs=wtout_sb[:],
                                     start=True, stop=True)
                    # ELU via TT (single-port DVE) + ACT only.
                    u = p2s.tile([128, 128], F32, tag="u")
                    nc.vector.tensor_add(out=u[:], in0=ps_o[:],
                                         in1=xloc[:, c, :])
                    nc.vector.tensor_add(out=u[:], in0=u[:], in1=bout_sb[:])
                    ng = p2s.tile([128, 128], F32, tag="ng")
                    nc.vector.tensor_tensor(out=ng[:], in0=u[:], in1=zero_b,
                                            op=mybir.AluOpType.min)
                    nc.scalar.activation(out=ng[:], in_=ng[:],
                                         func=mybir.ActivationFunctionType.Exp)
                    ps2 = p2s.tile([128, 128], F32, tag="ps2")
                    nc.scalar.activation(out=ps2[:], in_=u[:],
                                         func=mybir.ActivationFunctionType.Relu)
                    nc.vector.tensor_add(out=ps2[:], in0=ps2[:], in1=ng[:])
                    nc.vector.tensor_tensor(out=ps2[:], in0=ps2[:], in1=one_b,
                                            op=mybir.AluOpType.subtract)
                    eng = nc.sync if c % 2 == 0 else nc.scalar
                    eng.dma_start(out=y_out[sl, :], in_=ps2[:])

    nc.finalize()
    return nc


# --------------------------------------------------------------------------
# host-side prep
# --------------------------------------------------------------------------

def _host_weights(W_r, W_q, a_src, a_dst, W_out):
    d_h = np.abs(a_src).argmax(axis=1)
    kept = [(h, dh) for h in range(H) for dh in range(DH) if dh != d_h[h]]
    G = np.zeros((D, D), np.float64)
    for h in range(H):
        G[h * DH:(h + 1) * DH, h] = a_src[h]
    for jp, (h, dh) in enumerate(kept):
        G[DH * h + dh, 8 + jp] = 1.0
    Wt4 = np.concatenate([W_r[r].astype(np.float64) @ G for r in range(R)],
                         axis=1)
    Wtout = np.linalg.inv(G) @ W_out.astype(np.float64)
    Wd = np.zeros((D, H), np.float64)
    for h in range(H):
        Wd[:, h] = W_q[:, h * DH:(h + 1) * DH].astype(np.float64) @ a_dst[h]
    bf = ml_dtypes.bfloat16
    return Wt4.astype(bf), Wtout.astype(bf), Wd.astype(bf)


def _prep_inputs(inputs):
    feats = np.asarray(inputs["features"], np.float32)
    mask = np.asarray(inputs["mask"])
    edge_index = np.asarray(inputs["edge_index"], np.int64)
    edge_type = np.asarray(inputs["edge_type"], np.int64)
    speaker_ids = np.asarray(inputs["speaker_ids"], np.int64)
    spk_emb = np.asarray(inputs["spk_emb"], np.float32)
    W_r = np.asarray(inputs["W_r"], np.float32)
    W_q = np.asarray(inputs["W_q"], np.float32)
    a_src = np.asarray(inputs["a_src"], np.float32)
    a_dst = np.asarray(inputs["a_dst"], np.float32)
    W_out = np.asarray(inputs["W_out"], np.float32)
    b_out = np.asarray(inputs["b_out"], np.float32)
    bf = ml_dtypes.bfloat16

    Wt4, Wtout, Wd = _host_weights(W_r, W_q, a_src, a_dst, W_out)
    iota_row = np.arange(128, dtype=np.float32).astype(bf)
    spk_emb_b = spk_emb.astype(bf)

    cnt = np.clip(mask.astype(np.float32).sum(1), 1.0, None)
    w_tok = mask.astype(np.float32) / cnt[:, None]
    fw = (feats * w_tok[:, :, None]).astype(bf)

    src, tgt = edge_index[0], edge_index[1]
    core_of = tgt // NL
    loc_t = tgt - core_of * NL
    chunk_of = loc_t // 128
    t_in_chunk = loc_t % 128
    src_core = src // NL
    row_in_win = (src % NL) * R + edge_type

    key = (core_of * NT + chunk_of) * C + src_core
    order = np.argsort(key, kind="stable")
    key_s = key[order]
    row_s = row_in_win[order]
    tic_s = t_in_chunk[order]

    counts = np.bincount(key_s, minlength=C * NT * C).reshape(C, NT, C)
    useg = counts.max(axis=0)                       # [NT(chunk), C(window)]
    useg128 = ((useg + 127) // 128) * 128
    ncol_w = useg128 // 128                         # cols per (chunk, window)
    colcnt = [int(ncol_w[c].sum()) for c in range(NT)]
    MAXC = max(colcnt)

    starts = np.zeros(C * NT * C + 1, np.int64)
    np.cumsum(np.bincount(key_s, minlength=C * NT * C), out=starts[1:])

    # group sizes taper at the end so the pipeline drains faster
    sizes = []
    left = NT
    while left > GC + 2:
        sizes.append(GC)
        left -= GC
    while left > 0:
        s = max(1, min(2, left))
        sizes.append(s)
        left -= s
    bounds = np.concatenate([[0], np.cumsum(sizes)]).astype(int)

    groups = []
    off16_total = 0
    for gx in range(len(sizes)):
        cs = list(range(bounds[gx], bounds[gx + 1]))
        wmeta = []          # per window: (col0, n_idx, off16)
        cmap = {c: [] for c in cs}
        off_cols = 0
        for w in range(C):
            n_idx = int(sum(useg128[c, w] for c in cs))
            wmeta.append((off_cols, n_idx, off16_total))
            p = off_cols
            for c in cs:
                k = int(ncol_w[c, w])
                cmap[c].extend(range(p, p + k))
                p += k
            off_cols += n_idx // 128
            off16_total += n_idx // 16
        groups.append(dict(chunks=cs, wmeta=wmeta, cmap=cmap,
                           total_cols=off_cols))
    TOT16 = off16_total

    in_maps = []
    for ci in range(C):
        tl = np.full((NT, 128, MAXC), -1.0, np.float32)
        AT = np.zeros((NT, 128, MAXC * 128), np.uint8)
        gidx16 = np.zeros((128, TOT16), np.int16)
        for g in groups:
            for w in range(C):
                _, n_idx, off16 = g["wmeta"][w]
                vals = np.zeros(n_idx, np.int64)
                p = 0
                for c in g["chunks"]:
                    k = ci * NT * C + c * C + w
                    lo, hi = starts[k], starts[k + 1]
                    n = hi - lo
                    vals[p:p + n] = row_s[lo:hi]
                    base_col = int(ncol_w[c, :w].sum())
                    q = np.arange(n)
                    col = base_col + q // 128
                    prt = q % 128
                    tic = tic_s[lo:hi]
                    tl[c, prt, col] = tic
                    AT[c, tic, col * 128 + prt] = 0x38  # fp8e4m3 1.0
                    p += int(useg128[c, w])
                i = np.arange(n_idx)
                v16 = vals.astype(np.int16)
                for kq in range(8):
                    gidx16[i % 16 + 16 * kq, off16 + i // 16] = v16

        sl = slice(ci * NL, (ci + 1) * NL)
        featc = np.zeros((NLP, L, D), bf)
        featc[:NL] = fw[sl]
        spk1hT = np.zeros((16, NLP), bf)
        spk1hT[speaker_ids[sl], np.arange(NL)] = 1.0
        tlT = np.ascontiguousarray(
            tl.transpose(1, 0, 2).reshape(128, NT * MAXC)).astype(bf)

        in_maps.append(dict(
            feat=featc, spk1hT=spk1hT, spk_emb=spk_emb_b,
            Wt4=Wt4, Wd=Wd, Wtout=Wtout, b_out=b_out,
            iota_row=iota_row, gidx16=gidx16, tgtloc=tlT,
            ATH=AT.view(ml_dtypes.float8_e4m3fn),
        ))
    return in_maps, colcnt, MAXC, groups, TOT16


# --------------------------------------------------------------------------
# entry point
# --------------------------------------------------------------------------

def kernel(**inputs):
    _install_patch()
    import concourse.bass2jax as bass2jax

    in_maps, colcnt, MAXC, groups, TOT16 = _prep_inputs(inputs)
    nc = _build_kernel(colcnt, MAXC, groups, TOT16)
    results = bass2jax.run_bass_via_pjrt(nc, in_maps, n_cores=C)
    N = np.asarray(inputs["features"]).shape[0]
    NLc = N // C
    return np.concatenate(
        [np.asarray(results[c]["y"])[:NLc] for c in range(C)], axis=0
    ).astype(np.float32)
